# revision 25
# baseline (speedup 1.0000x reference)
"""Sliding-window causal GQA attention block (QKV proj + RoPE + SDPA + out proj)
on 8 Trainium2 NeuronCores.

Sharding: 8 cores = 2 batches x 4 sequence chunks of 512 tokens. Each core
computes the full attention-block output for its (batch, seq-chunk):
  - Q projection for its 512 queries (all 16 heads) in transposed [d, s] layout
  - K/V projection for its chunk + 512-token halo (sliding window support)
  - RoPE via rotate-half permutation matmul + element-wise mul/add
  - sliding-window causal attention with scores kept transposed [keys, queries]
    so no on-chip transposes are needed (fp32 has no DMA transpose on trn2)
  - softmax denominators via ones-vector matmul (partition-dim reduction)
  - full out-projection computed transposed (y^T = wo^T-tiles @ o^T), no
    inter-core reduction needed; host transposes each core's 4MB slab back.

Matmul operands are bf16 (x and weights pre-cast on the host, activations cast
by the producing engine's output dtype); all accumulation is fp32 in PSUM, and
softmax denominators/reciprocals are fp32. Measured end-to-end max relative
error vs the fp32 reference is ~2.6e-3 (rms ~5e-4).
"""
import numpy as np

import concourse.bacc as bacc
import concourse.mybir as mybir
import concourse.tile as tile
from concourse.bass_utils import run_bass_kernel_spmd

# Problem constants (hardcoded per contract)
B, S, E = 2, 2048, 2048
H, KV, D = 16, 4, 128
WIN = 512
THETA = 1e6
NCORES = 8
CH = 512          # seq chunk per core
SW = 1024         # K/V window per core (halo 512 + own 512)
P = 128
ECH = E // P      # 16 contraction chunks
NJT = SW // P     # 8 key tiles in window
F32 = mybir.dt.float32
BF16 = mybir.dt.bfloat16
SCALE = 1.0 / float(np.sqrt(np.float32(D)))

_CACHE = {}


def _build():
    nc = bacc.Bacc("TRN2", target_bir_lowering=False, debug=False,
                   num_devices=NCORES)

    xt = nc.dram_tensor("xt", [E, SW], BF16, kind="ExternalInput")
    wqkv = nc.dram_tensor("wqkv", [E, (H + 2 * KV) * D], BF16, kind="ExternalInput")
    wo = nc.dram_tensor("wo", [H * D, E], BF16, kind="ExternalInput")
    cosw = nc.dram_tensor("cosw", [P, SW], F32, kind="ExternalInput")
    sinw = nc.dram_tensor("sinw", [P, SW], F32, kind="ExternalInput")
    masks = nc.dram_tensor("masks", [12, P, CH], BF16, kind="ExternalInput")
    perm = nc.dram_tensor("perm", [P, P], BF16, kind="ExternalInput")
    ones = nc.dram_tensor("ones", [1, P], BF16, kind="ExternalInput")
    yt = nc.dram_tensor("yt", [E, CH], F32, kind="ExternalOutput")

    KOFF = H * D            # w_qkv column offsets
    VOFF = H * D + KV * D

    with tile.TileContext(nc) as tc:
        with (
            tc.tile_pool(name="res", bufs=1) as res,       # resident tensors
            tc.tile_pool(name="big", bufs=2) as big,       # x_halo/wv then o_T
            tc.tile_pool(name="wst", bufs=4) as wst,       # streamed w tiles
            tc.tile_pool(name="tmp", bufs=3) as tmp,       # transient compute
            tc.tile_pool(name="pj", bufs=2, space="PSUM") as pj,
            tc.tile_pool(name="ps1", bufs=2, space="PSUM") as ps1,  # scores
            tc.tile_pool(name="ps2", bufs=2, space="PSUM") as ps2,  # av
            tc.tile_pool(name="psd", bufs=2, space="PSUM") as psd,  # denom
        ):
            # ---------------- constants (gpsimd queue) ----------------------
            cos_sb = res.tile([P, SW], F32, tag="cosw")
            sin_sb = res.tile([P, SW], F32, tag="sinw")
            nc.gpsimd.dma_start(cos_sb[:], cosw.ap())
            nc.gpsimd.dma_start(sin_sb[:], sinw.ap())
            perm_sb = res.tile([P, P], BF16, tag="perm")
            nc.gpsimd.dma_start(perm_sb[:], perm.ap())
            ones_sb = res.tile([P, 1], BF16, tag="ones")
            nc.gpsimd.dma_start(ones_sb[:], ones.ap().rearrange("o p -> p o"))
            mask_sb = res.tile([P, 12, CH], BF16, tag="masks")
            for mi in range(12):
                nc.gpsimd.dma_start(mask_sb[:, mi, :], masks.ap()[mi])

            # ------------- x into SBUF (bf16, host pre-cast; HWDGE) ----------
            x_own = res.tile([P, ECH, CH], BF16, tag="xown")
            x_halo = big.tile([P, ECH, CH], BF16, tag="big")
            xt3 = xt.ap().rearrange("(eo p) s -> p eo s", p=P)
            wqkv3 = wqkv.ap().rearrange("(eo p) f -> p eo f", p=P)
            wk_t = {}

            def load_wk(fk):
                wk_t[fk] = wst.tile([P, ECH, P], BF16, tag="wqk",
                                    name=f"wk_{fk}")
                for eh in range(2):
                    sl = slice(eh * 8, eh * 8 + 8)
                    nc.sync.dma_start(
                        wk_t[fk][:, sl, :],
                        wqkv3[:, sl, KOFF + fk * P:KOFF + (fk + 1) * P])

            load_wk(0)
            for eh in range(4):
                sl = slice(eh * 4, eh * 4 + 4)
                nc.sync.dma_start(x_halo[:, sl, :], xt3[:, sl, 0:CH])
            for eh in range(4):
                sl = slice(eh * 4, eh * 4 + 4)
                nc.sync.dma_start(x_own[:, sl, :], xt3[:, sl, CH:SW])

            def x_win_slice(e, st):
                """lhsT [128 e-part, 128 s-cols] for window s-tile st (0..7)."""
                if st < 4:
                    return x_halo[:, e, st * P:(st + 1) * P]
                return x_own[:, e, (st - 4) * P:(st - 3) * P]

            # ---------------- rope helper ----------------
            def rope(dst, raw_ps, c0, c1, split2=False):
                """dst[128, n] = rope(raw) using cos/sin window cols [c0:c1)."""
                n = c1 - c0
                raw_sb = tmp.tile([P, CH], BF16, tag="qraw")
                nc.vector.tensor_copy(out=raw_sb[:, :n], in_=raw_ps[:, :n])
                rot_ps = ps1.tile([P, CH], F32, tag="sc")
                nc.tensor.matmul(rot_ps[:, :n], perm_sb[:], raw_sb[:, :n],
                                 start=True, stop=True)
                t1 = tmp.tile([P, CH], F32, tag="qraw")
                nc.gpsimd.tensor_mul(out=t1[:, :n], in0=raw_sb[:, :n],
                                     in1=cos_sb[:, c0:c1])
                t2 = tmp.tile([P, CH], F32, tag="qraw")
                nc.vector.tensor_mul(out=t2[:, :n], in0=rot_ps[:, :n],
                                     in1=sin_sb[:, c0:c1])
                if split2:
                    nc.vector.tensor_add(
                        out=dst,
                        in0=t1[:, :n].rearrange("p (a b) -> p a b", a=2),
                        in1=t2[:, :n].rearrange("p (a b) -> p a b", a=2))
                else:
                    nc.vector.tensor_add(out=dst, in0=t1[:, :n], in1=t2[:, :n])

            # ------------- K projection (transposed [d, s] layout) -----------
            k_sb = res.tile([P, KV, SW], BF16, tag="k")
            for fk in range(KV):
                if fk > 0:
                    load_wk(fk)
                for sh in range(SW // CH):
                    k_ps = pj.tile([P, CH], F32, tag="pj")
                    for e in range(ECH):
                        nc.tensor.matmul(
                            k_ps[:], wk_t[fk][:, e, :],
                            (x_halo if sh == 0 else x_own)[:, e, :],
                            start=(e == 0), stop=(e == ECH - 1))
                    rope(k_sb[:, fk, sh * CH:(sh + 1) * CH], k_ps,
                         sh * CH, (sh + 1) * CH)

            # ------------- Q projection (transposed [d, s] layout) -----------
            # q_sb free layout: block blk = kv*4 + hp*2 + p2 (16 blocks of 512);
            # within a block: [head-sub 0 | head-sub 1] x 256 queries.
            # Head fi = 4*kv + 2*hp + sub owns columns sub*256..sub*256+256 of
            # blocks blk0 = kv*4+hp*2 (p2=0) and blk0+1 (p2=1).
            q_sb = res.tile([P, 16, CH], BF16, tag="q")
            for fi in range(H):
                wq_t = wst.tile([P, ECH, P], BF16, tag="wqk")
                for eh in range(2):
                    sl = slice(eh * 8, eh * 8 + 8)
                    nc.sync.dma_start(wq_t[:, sl, :],
                                        wqkv3[:, sl, fi * P:(fi + 1) * P])
                q_ps = pj.tile([P, CH], F32, tag="pj")
                for e in range(ECH):
                    nc.tensor.matmul(q_ps[:], wq_t[:, e, :], x_own[:, e, :],
                                     start=(e == 0), stop=(e == ECH - 1))
                kvb, hp, sub = fi // 4, (fi % 4) // 2, fi % 2
                blk0 = kvb * 4 + hp * 2
                dst = q_sb[:, blk0:blk0 + 2, sub * 256:sub * 256 + 256]
                rope(dst, q_ps, CH, SW, split2=True)

            # wv resident in one big-pool slot, [p, e_chunk, v_cols 512]
            wv_sb = big.tile([P, ECH, KV * D], BF16, tag="big")
            for eh in range(4):
                sl = slice(eh * 4, eh * 4 + 4)
                nc.gpsimd.dma_start(wv_sb[:, sl, :],
                                    wqkv3[:, sl, VOFF:VOFF + KV * D])

            # ------------- V projection (natural [s, d] layout) --------------
            v_sb = res.tile([P, NJT, KV * D], BF16, tag="v")
            for st in range(NJT):
                v_ps = pj.tile([P, KV * D], F32, tag="pj")
                for e in range(ECH):
                    nc.tensor.matmul(v_ps[:], x_win_slice(e, st), wv_sb[:, e, :],
                                     start=(e == 0), stop=(e == ECH - 1))
                nc.vector.tensor_copy(out=v_sb[:, st, :], in_=v_ps[:])

            # ---- attention: head-pairs sharing a kv head, batched to N=512 --
            # rhs block = [head-a 256 queries | head-b 256 queries] of pair p2.
            o_sb = big.tile([P, 16, CH], BF16, tag="big")
            for kvb in range(KV):
                for hp in range(2):
                    for p2 in range(2):
                        blk = kvb * 4 + hp * 2 + p2
                        av_ps = ps2.tile([P, CH], F32, tag="av")
                        dn_ps = psd.tile([1, CH], F32, tag="dn")
                        for r in range(6):
                            jt = 2 * p2 + r
                            scpool, sctag = (ps1, "sc") if r % 2 else (pj, "pj")
                            sc_ps = scpool.tile([P, CH], F32, tag=sctag,
                                                name=f"sc_{kvb}_{hp}_{p2}_{r}")
                            nc.tensor.matmul(sc_ps[:],
                                             k_sb[:, kvb, jt * P:(jt + 1) * P],
                                             q_sb[:, blk, :],
                                             start=True, stop=True)
                            pe = tmp.tile([P, CH], BF16, tag="pe")
                            nc.scalar.activation(
                                out=pe[:], in_=sc_ps[:],
                                func=mybir.ActivationFunctionType.Exp,
                                scale=SCALE)
                            pt = tmp.tile([P, CH], BF16, tag="pt")
                            nc.vector.tensor_mul(out=pt[:], in0=pe[:],
                                                 in1=mask_sb[:, p2 * 6 + r, :])
                            nc.tensor.matmul(av_ps[:],
                                             v_sb[:, jt, kvb * D:(kvb + 1) * D],
                                             pt[:], start=(r == 0),
                                             stop=(r == 5))
                            nc.tensor.matmul(dn_ps[:], ones_sb[:], pt[:],
                                             start=(r == 0), stop=(r == 5))
                        den = tmp.tile([1, CH], F32, tag="den")
                        nc.vector.tensor_copy(out=den[:], in_=dn_ps[:])
                        bc = tmp.tile([P, CH], F32, tag="bc")
                        nc.gpsimd.partition_broadcast(bc[:], den[:])
                        rc = tmp.tile([P, CH], F32, tag="rc")
                        nc.vector.reciprocal_approx_fast(out=rc[:], in_=bc[:])
                        nc.vector.tensor_mul(out=o_sb[:, blk, :],
                                             in0=av_ps[:], in1=rc[:])

            # ------------- out projection, transposed: yt = sum_f woT @ oT ---
            wo3 = wo.ap().rearrange("(fo p) e2 -> p fo e2", p=P)
            for et in range(ECH):
                wo_t = wst.tile([P, ECH, P], BF16, tag="wo")
                for fh in range(2):
                    sl = slice(fh * 8, fh * 8 + 8)
                    nc.sync.dma_start(
                        wo_t[:, sl, :], wo3[:, sl, et * P:(et + 1) * P])
                y_ps = pj.tile([P, CH], F32, tag="pj")
                for f in range(H):
                    kvb, hp, sub = f // 4, (f % 4) // 2, f % 2
                    blk0 = kvb * 4 + hp * 2
                    o_f = o_sb[:, blk0:blk0 + 2, sub * 256:sub * 256 + 256]
                    nc.tensor.matmul(y_ps[:], wo_t[:, f, :], o_f,
                                     start=(f == 0), stop=(f == H - 1))
                y_sb = tmp.tile([P, CH], F32, tag="ysb")
                nc.vector.tensor_copy(out=y_sb[:], in_=y_ps[:])
                nc.sync.dma_start(yt.ap()[et * P:(et + 1) * P, :], y_sb[:])

    nc.compile()
    return nc


def _host_constants():
    inv_freq = (1.0 / (THETA ** (np.arange(0, D, 2, dtype=np.float32) / D))
                ).astype(np.float32)
    ang = np.arange(S, dtype=np.float32)[:, None] * inv_freq[None, :]
    emb = np.concatenate([ang, ang], axis=-1)          # [S, D]
    cos_t = np.ascontiguousarray(np.cos(emb).astype(np.float32).T)  # [D, S]
    sin_t = np.ascontiguousarray(np.sin(emb).astype(np.float32).T)

    import ml_dtypes
    pm = np.zeros((P, P), dtype=np.float32)            # rotate-half as lhsT
    a = np.arange(64)
    pm[a, a + 64] = 1.0
    pm[a + 64, a] = -1.0
    pm = pm.astype(ml_dtypes.bfloat16)

    onesv = np.ones((1, P), dtype=ml_dtypes.bfloat16)
    return cos_t, sin_t, pm, onesv


def _masks_for_chunk(chunk):
    """[12, 128, 512] bf16: mask[p2*6+r, jj, :] for head-pair blocks.

    Columns are [head-a 256 queries | head-b 256 queries] of pair p2; the
    mask depends only on the query position, so the two halves are equal."""
    import ml_dtypes
    m = np.zeros((12, P, CH), dtype=np.float32)
    s0 = chunk * CH
    for p2 in range(2):
        q_glob = s0 + p2 * 256 + np.arange(256)[None, :]
        for r in range(6):
            jt = 2 * p2 + r
            jg0 = s0 - WIN + jt * P
            j_glob = jg0 + np.arange(P)[:, None]
            dlt = q_glob - j_glob
            ok = ((dlt >= 0) & (dlt < WIN) & (j_glob >= 0)).astype(np.float32)
            m[p2 * 6 + r] = np.concatenate([ok, ok], axis=1)
    return m.astype(ml_dtypes.bfloat16)


def _prepare_in_maps(x, w_qkv, w_o):
    import ml_dtypes
    cos_t, sin_t, pm, onesv = _host_constants()
    w_qkv = np.ascontiguousarray(w_qkv, dtype=np.float32).astype(ml_dtypes.bfloat16)
    w_o = np.ascontiguousarray(w_o, dtype=np.float32).astype(ml_dtypes.bfloat16)
    in_maps = []
    xts = [np.ascontiguousarray(np.asarray(x[b], dtype=np.float32).T
                                ).astype(ml_dtypes.bfloat16)
           for b in range(B)]
    for c in range(NCORES):
        b, chunk = divmod(c, 4)
        s0 = chunk * CH
        xt_win = np.zeros((E, SW), dtype=ml_dtypes.bfloat16)
        cos_win = np.zeros((P, SW), dtype=np.float32)
        sin_win = np.zeros((P, SW), dtype=np.float32)
        lo = s0 - WIN
        src_lo = max(0, lo)
        dst_lo = src_lo - lo
        xt_win[:, dst_lo:] = xts[b][:, src_lo:s0 + CH]
        cos_win[:, dst_lo:] = cos_t[:, src_lo:s0 + CH]
        sin_win[:, dst_lo:] = sin_t[:, src_lo:s0 + CH]
        in_maps.append({
            "xt": xt_win,
            "wqkv": w_qkv,
            "wo": w_o,
            "cosw": cos_win,
            "sinw": sin_win,
            "masks": _masks_for_chunk(chunk),
            "perm": pm,
            "ones": onesv,
        })
    return in_maps


def _install_ntff_shim():
    """bass_utils wants antenv.axon_hooks for trace=True under axon; this
    environment lacks that module, so synthesize it from the boot helper."""
    import sys
    import types
    if "antenv.axon_hooks" in sys.modules:
        return
    try:
        from trn_agent_boot.trn_boot import _ntff_profile_via_ctypes
        hook = _ntff_profile_via_ctypes("/opt/axon/libaxon_pjrt.so")
    except Exception:
        hook = None
    mod = types.ModuleType("antenv.axon_hooks")
    mod.get_axon_ntff_profile_hook = lambda: hook
    mod.set_axon_ntff_profile_hook = lambda h: None
    sys.modules["antenv.axon_hooks"] = mod


def run(x, w_qkv, w_o, trace=False):
    if "nc" not in _CACHE:
        _CACHE["nc"] = _build()
    nc = _CACHE["nc"]
    in_maps = _prepare_in_maps(np.asarray(x), np.asarray(w_qkv),
                               np.asarray(w_o))
    if trace:
        _install_ntff_shim()
    try:
        res = run_bass_kernel_spmd(nc, in_maps, list(range(NCORES)),
                                   trace=trace)
    except Exception:
        if not trace:
            raise
        res = run_bass_kernel_spmd(nc, in_maps, list(range(NCORES)),
                                   trace=False)
    y = np.empty((B, S, E), dtype=np.float32)
    for c in range(NCORES):
        b, chunk = divmod(c, 4)
        y[b, chunk * CH:(chunk + 1) * CH, :] = res.results[c]["yt"].T
    return y, res


def kernel(x, w_qkv, w_o):
    y, _ = run(x, w_qkv, w_o, trace=False)
    return y
